# revision 16
# baseline (speedup 1.0000x reference)
"""RNN-T JointNetwork kernel for 8 Trainium2 NeuronCores.

Math: out[b,t,u,:] = tanh(concat(fe[b,t], gd[b,u])) @ Wj + bj
with fe = f@We+be, gd = g@Wd+bd.

Since tanh acts elementwise and the concat feeds a single GEMM, the joint
GEMM factorizes exactly:
    out[b,t,u,:] = A[b,t,:] + C[b,u,:]
    A = tanh(f@We+be) @ Wj[:Dm]          (per-(b,t) row)
    C = tanh(g@Wd+bd) @ Wj[Dm:] + bj     (per-(b,u) row)
This collapses the 137-GFLOP joint GEMM into two tiny GEMMs plus a
broadcast-add, leaving the kernel bound by the output write.

Sharding: 8 cores, core c owns (b = c//2, t-half = c%2) -> a [128,64,V]
output chunk per core.

Trace-driven design (profiled on trn2):
  - HBM reads cap ~290 GB/s (latency-bound; writes sustain ~440) and
    DMAs on one queue complete near-FIFO, so inputs stream in priority
    order: pack_g (g-path gates the longest chain) -> pack_f ->
    wjb_v0 -> wjt_v0 -> sel -> wjb_v1 -> wjt_v1.
  - Everything is pipelined by v-half: the v0 weights land first, the
    v0 half of ACp is built, and a wave of 8 superchunks streams v0
    output while the v1 weights arrive and the v1 prologue GEMMs slot
    between wave superchunks; then the v1 wave.
  - Both ACp tiles use the SAME layout [A-half ; C] so ONE selector
    serves all 16 superchunks.  The A halves land in partitions 0:64
    of separate psum tiles via column-sliced stationary operands; C is
    computed once at partitions 64:128 and copied into both tiles.
  - Superchunk output: partition p owns out rows 8p..8p+8 of a 1024-row
    block = one 8 KB contiguous DRAM run.  The DMA must see a flat 2D
    AP: a [128,8,512] 3-dim view of the same bytes measured ~342 GB/s
    vs ~395-440 for the 2D slice.  Output DRAM is vh-major
    [2*8192, 512]; host reassembles the v halves.
  - psO->SBUF copies only run on ACT (1.11us/[128,1024]) and DVE
    (1.22us) - GpSimd cannot read PSUM on trn2.  Engine is FIXED per
    psum ring slot, and the v1-prologue psum allocations are padded to
    a full ring rotation: an unpadded injection rotates the ring phase
    so every superchunk's first matmul waits on the PREVIOUS
    superchunk's last copy (measured 3.0us/superchunk vs ~2.5).
  - The PE HAM clock gate idles back to ~1.1 GHz after ~1 us of
    inactivity and needs several us of sustained matmuls to re-ramp:
    dummy matmuls bridge the input-wait gaps in the prologue so the
    real GEMMs run near 2.4 GHz (measured 0.63us -> 0.38us / 512 cols).
  - out is bf16: tolerance 2e-2 dwarfs bf16's ~5e-3; host upcasts.
"""

import sys

sys.path.insert(0, "/opt/trn_rl_repo")

import numpy as np

import concourse.bacc as bacc
import concourse.mybir as mybir
import concourse.tile as tile
from concourse.bass_utils import run_bass_kernel_spmd

B, T, U = 4, 256, 64
D = 512  # DE = DD = DM
V = 1024
TC = 128  # t rows per core
NCORES = 8
FP32 = mybir.dt.float32
BF16 = mybir.dt.bfloat16
NPBF16 = mybir.dt.np(mybir.dt.bfloat16)
FP8 = mybir.dt.float8e4
NPFP8 = mybir.dt.np(mybir.dt.float8e4)
TANH = mybir.ActivationFunctionType.Tanh

# pack_g: gT | Wd | biases ; pack_f: fT | We
OFF_GT, OFF_WD, OFF_B = 0, 256, 2304
PACKG_COLS = 2312
OFF_FT, OFF_WE = 0, 512
PACKF_COLS = 2560

_cache = {}


def _build_nc():
    nc = bacc.Bacc("TRN2", target_bir_lowering=False)

    packg_d = nc.dram_tensor("packg", [128, PACKG_COLS], BF16, kind="ExternalInput")
    packf_d = nc.dram_tensor("packf", [128, PACKF_COLS], BF16, kind="ExternalInput")
    brow_d = nc.dram_tensor("brow", [1, V + 64], BF16, kind="ExternalInput")
    wjbv_d = [nc.dram_tensor(f"wjb{v}", [128, 2048], BF16, kind="ExternalInput") for v in range(2)]
    wjtv_d = [nc.dram_tensor(f"wjt{v}", [128, 2048], BF16, kind="ExternalInput") for v in range(2)]
    sel_d = nc.dram_tensor("sel", [128, 4096], FP8, kind="ExternalInput")
    # vh-major: flat row vh*8192 + F holds v-cols vh*512..+512 of out
    # flat row F
    out_d = nc.dram_tensor("out", [2 * 8192, 512], BF16, kind="ExternalOutput")

    with tile.TileContext(nc) as tc:
        with tc.tile_pool(name="wts", bufs=1) as wp:
            packg = wp.tile([128, PACKG_COLS], BF16, tag="packg")
            packf = wp.tile([128, PACKF_COLS], BF16, tag="packf")
            brow = wp.tile([1, V + 64], BF16, tag="brow")
            wjbv = [wp.tile([128, 2048], BF16, tag=f"wjb{v}", name=f"wjb{v}") for v in range(2)]
            wjtv = [wp.tile([128, 2048], BF16, tag=f"wjt{v}", name=f"wjt{v}") for v in range(2)]
            sel = wp.tile([128, 4096], FP8, tag="sel")
            tfT = [wp.tile([128, TC], BF16, tag=f"tfT{c}", name=f"tfT{c}") for c in range(4)]
            tgT = [wp.tile([128, U], BF16, tag=f"tgT{c}", name=f"tgT{c}") for c in range(4)]
            ACp0 = wp.tile([128, V], BF16, tag="ACp0")
            ACp1 = wp.tile([128, V], BF16, tag="ACp1")

            # input stream: arrival priority = issue order (near-FIFO)
            nc.sync.dma_start(packg[:], packg_d[:])
            nc.sync.dma_start(brow[:], brow_d[:])
            nc.sync.dma_start(packf[:], packf_d[:])
            nc.sync.dma_start(wjbv[0][:], wjbv_d[0][:])
            nc.sync.dma_start(wjtv[0][:], wjtv_d[0][:])
            nc.sync.dma_start(sel[:], sel_d[:])
            nc.sync.dma_start(wjbv[1][:], wjbv_d[1][:])
            nc.sync.dma_start(wjtv[1][:], wjtv_d[1][:])

            fT = [packf[:, OFF_FT + c * 128 : OFF_FT + (c + 1) * 128] for c in range(4)]
            We = [packf[:, OFF_WE + c * 512 : OFF_WE + (c + 1) * 512] for c in range(4)]
            gT = [packg[:, OFF_GT + c * 64 : OFF_GT + (c + 1) * 64] for c in range(4)]
            Wd = [packg[:, OFF_WD + c * 512 : OFF_WD + (c + 1) * 512] for c in range(4)]
            be = lambda mc: packg[:, OFF_B + mc : OFF_B + mc + 1]
            bd = lambda mc: packg[:, OFF_B + 4 + mc : OFF_B + 5 + mc]
            wj_b = lambda mc, vh: wjbv[vh][:, 512 * mc : 512 * mc + 512]
            wj_t = lambda mc, vh: wjtv[vh][:, 512 * mc : 512 * mc + 512]
            ones64 = brow[:, V : V + 64]
            bj = lambda vh: brow[:, vh * 512 : (vh + 1) * 512]

            def c_gemm(ps, vh):
                """C v-half into psum partitions 64:128 (+bj)."""
                for mc in range(4):
                    nc.tensor.matmul(
                        ps[64:128, :], tgT[mc][:], wj_b(mc, vh),
                        start=(mc == 0), stop=False,
                    )
                nc.tensor.matmul(
                    ps[64:128, :], ones64, bj(vh), start=False, stop=True
                )

            def a_gemm(ps, h, vh):
                """A t-half h, v-half vh into psum partitions 0:64."""
                hs = slice(h * 64, (h + 1) * 64)
                for mc in range(4):
                    nc.tensor.matmul(
                        ps[0:64, :], tfT[mc][:, hs], wj_t(mc, vh),
                        start=(mc == 0), stop=(mc == 3),
                    )

            def c_copies(ps, vh):
                vs = slice(vh * 512, (vh + 1) * 512)
                nc.scalar.copy(ACp0[64:128, vs], ps[64:128, :])
                nc.vector.tensor_copy(ACp1[64:128, vs], ps[64:128, :])

            def a_copy(ps, h, vh):
                vs = slice(vh * 512, (vh + 1) * 512)
                acp = (ACp0, ACp1)[h]
                if h == 0:
                    nc.scalar.copy(acp[0:64, vs], ps[0:64, :])
                else:
                    nc.vector.tensor_copy(acp[0:64, vs], ps[0:64, :])

            # ---- v0 prologue ----
            with tc.tile_pool(name="pp", bufs=1, space="PSUM") as pp:
                # PE warm-up; results never read.  Interleaved with the
                # real GEMM groups so the HAM clock never idles down
                # during input-wait gaps (see module doc).
                scratch = wp.tile([128, 640], BF16, tag="scratch")
                nc.vector.memset(scratch[:], 1.0)
                wps = pp.tile([128, 512], FP32, tag="pps", bufs=4)

                def warm(n):
                    for _ in range(n):
                        nc.tensor.matmul(
                            wps[:], scratch[:, 0:128], scratch[:, 128:640],
                            start=True, stop=True,
                        )

                warm(4)
                # g-path first: C's dependency chain is longest
                for mc in range(4):
                    ms = slice(mc * 128, (mc + 1) * 128)
                    ps = pp.tile([128, U], FP32, tag="pps", bufs=4)
                    for dc in range(4):
                        nc.tensor.matmul(
                            ps[:], Wd[dc][:, ms], gT[dc],
                            start=(dc == 0), stop=(dc == 3),
                        )
                    nc.scalar.activation(tgT[mc][:], ps[:], TANH, bias=bd(mc))
                warm(3)  # bridge the wait for packf
                for mc in range(4):
                    ms = slice(mc * 128, (mc + 1) * 128)
                    ps = pp.tile([128, TC], FP32, tag="pps", bufs=4)
                    for dc in range(4):
                        nc.tensor.matmul(
                            ps[:], We[dc][:, ms], fT[dc],
                            start=(dc == 0), stop=(dc == 3),
                        )
                    nc.scalar.activation(tfT[mc][:], ps[:], TANH, bias=be(mc))
                warm(2)  # bridge the wait for wjb_v0

                psC0 = pp.tile([128, 512], FP32, tag="pj", bufs=3)
                c_gemm(psC0, 0)
                c_copies(psC0, 0)
                warm(1)  # bridge the wait for wjt_v0
                psA00 = pp.tile([128, 512], FP32, tag="pj", bufs=3)
                a_gemm(psA00, 0, 0)
                a_copy(psA00, 0, 0)
                psA10 = pp.tile([128, 512], FP32, tag="pj", bufs=3)
                a_gemm(psA10, 1, 0)
                a_copy(psA10, 1, 0)

            # ---- main loop: 2 waves x 8 superchunks of [1024 rows, 512]
            # superchunk (vh, J), psum slot q: psO_q[p,:] = v-cols
            # vh*512..+512 of out flat row 1024J + 8p + q
            #   -> t = 16J + p//8, u = 8*(p%8) + q
            with (
                tc.tile_pool(name="po", bufs=1, space="PSUM") as po,
                tc.tile_pool(name="ob", bufs=5) as ob,
            ):
                def pot():
                    return po.tile([128, 1024], FP32, tag="psO", bufs=4, name="psO")

                for vh in range(2):
                    for J in range(8):
                        first = vh == 0 and J == 0
                        last = vh == 1 and J == 7
                        JJ = J % 4
                        acp = (ACp0, ACp1)[J // 4]
                        vs = slice(vh * 512, (vh + 1) * 512)
                        r0 = 8192 * vh + 1024 * J
                        out_sb = ob.tile([128, 4096], BF16, tag="out")
                        for k in range(4):
                            psO = pot()
                            for i in range(2):
                                q = 2 * k + i
                                c0 = 128 * (8 * JJ + q)
                                nc.tensor.matmul(
                                    psO[:, i * 512 : (i + 1) * 512],
                                    sel[:, c0 : c0 + 128],
                                    acp[:, vs],
                                    start=True, stop=True,
                                )
                            dst = out_sb[:, k * 1024 : (k + 1) * 1024]
                            # engine FIXED per slot k (see module doc)
                            if k % 2 == 0:
                                nc.scalar.copy(dst, psO[:])
                            else:
                                nc.vector.tensor_copy(dst, psO[:])
                            if (first or last) and k == 1:
                                # first/last superchunk ship in q-halves
                                # (earlier first write / shorter drain);
                                # 4 KB strided runs via a rearranged AP
                                ov = out_d[r0 : r0 + 1024, :].rearrange(
                                    "(p q) v -> p (q v)", q=8
                                )
                                nc.sync.dma_start(
                                    ov[:, 0:2048], out_sb[:, 0:2048]
                                )
                        if first or last:
                            ov = out_d[r0 : r0 + 1024, :].rearrange(
                                "(p q) v -> p (q v)", q=8
                            )
                            nc.sync.dma_start(
                                ov[:, 2048:4096], out_sb[:, 2048:4096]
                            )
                        else:
                            nc.sync.dma_start(
                                out_d[r0 : r0 + 1024, :], out_sb[:]
                            )

                        # v1 prologue GEMMs slot between early v0-wave
                        # superchunks once wj_v1 has landed; each
                        # injection pads to a FULL psO ring rotation so
                        # the slot->engine phase is preserved
                        if vh == 0 and J in (1, 3, 5):
                            aux = pot()[:, 0:512]
                            if J == 1:
                                c_gemm(aux, 1)
                                c_copies(aux, 1)
                            elif J == 3:
                                a_gemm(aux, 0, 1)
                                a_copy(aux, 0, 1)
                            else:
                                a_gemm(aux, 1, 1)
                                a_copy(aux, 1, 1)
                            for _ in range(3):
                                pot()  # phase padding, never touched

    nc.compile()
    return nc


def _chunkcat(M):
    """[N*128, C] -> [128, N*C]: stack 128-row chunks side by side."""
    n = M.shape[0] // 128
    return np.ascontiguousarray(
        M.reshape(n, 128, M.shape[1]).transpose(1, 0, 2).reshape(128, -1)
    )


def _build_selector():
    """Row-permuted pair selector (see main-loop comment)."""
    sel = np.zeros((128, 4096), np.float32)
    p = np.arange(128)
    for JJ in range(4):
        for q in range(8):
            col = 128 * (8 * JJ + q) + p
            sel[16 * JJ + p // 8, col] = 1.0
            sel[64 + 8 * (p % 8) + q, col] = 1.0
    return sel.astype(NPFP8)


def kernel(f, g, We, be, Wd, bd, Wj, bj):
    if "nc" not in _cache:
        _cache["nc"] = _build_nc()
    nc = _cache["nc"]

    b16 = lambda x: np.asarray(x, dtype=np.float32).astype(NPBF16)
    f = np.asarray(f, dtype=np.float32)
    g = np.asarray(g, dtype=np.float32)
    Wj = np.asarray(Wj, dtype=np.float32)

    sel = _build_selector()
    wjt = _chunkcat(b16(Wj[:D])).reshape(128, 4, 2, 512)  # [p, mc, vh, v]
    wjb = _chunkcat(b16(Wj[D:])).reshape(128, 4, 2, 512)
    brow = np.zeros((1, V + 64), np.float32)
    brow[0, :V] = np.asarray(bj, dtype=np.float32)
    brow[0, V:] = 1.0
    bias8 = np.zeros((128, 8), np.float32)
    for c in range(4):
        bias8[:, c] = np.asarray(be, dtype=np.float32)[c * 128 : (c + 1) * 128]
        bias8[:, 4 + c] = np.asarray(bd, dtype=np.float32)[c * 128 : (c + 1) * 128]
    We_p = _chunkcat(b16(We))
    Wd_p = _chunkcat(b16(Wd))

    shared = {"sel": sel, "brow": b16(brow)}
    for v in range(2):
        shared[f"wjt{v}"] = np.ascontiguousarray(wjt[:, :, v, :].reshape(128, 2048))
        shared[f"wjb{v}"] = np.ascontiguousarray(wjb[:, :, v, :].reshape(128, 2048))
    in_maps = []
    for c in range(NCORES):
        b, th = c // 2, c % 2
        fTp = _chunkcat(b16(f[b, th * TC : (th + 1) * TC, :].T))
        gTp = _chunkcat(b16(g[b].T))
        packg = np.concatenate([gTp, Wd_p, b16(bias8)], axis=1)
        packf = np.concatenate([fTp, We_p], axis=1)
        in_maps.append({
            "packg": np.ascontiguousarray(packg),
            "packf": np.ascontiguousarray(packf),
            **shared,
        })
    res = run_bass_kernel_spmd(nc, in_maps, list(range(NCORES)))
    kernel._last_results = res

    out = np.empty((B, T, U, V), np.float32)
    for c in range(NCORES):
        b, th = c // 2, c % 2
        raw = res.results[c]["out"].astype(np.float32).reshape(2 * 8192, 512)
        half = np.empty((8192, V), np.float32)
        half[:, 0:512] = raw[0:8192]
        half[:, 512:1024] = raw[8192:16384]
        out[b, th * TC : (th + 1) * TC] = half.reshape(TC, U, V)
    return out
